# revision 16
# baseline (speedup 1.0000x reference)
"""2-layer dense GCN on 8 Trainium2 NeuronCores.

Reference computation (all fp32):
    H0 = relu((A_norm @ X) @ W0)
    H1 = relu((A_norm @ H0) @ W1)
A_norm: [16384, 16384], X: [16384, 128], W0/W1: [128, 128].

Sharding: 1D row partition of A_norm (2048 rows/core). Each core holds
A[rows_c].T (host-transposed so the node-contraction dim lands on SBUF
partitions), computes its row block of each layer, and the hidden state
is exchanged between layers with chunked on-device AllGathers.

Device layout is transpose-free:
  - aggregate:  psum[d, i] += X_tile[j, d].T @ A_T_tile[j, i]
                (lhsT = stationary node-major X/H tile, rhs = A^T slice)
  - linear:     psum[i, e]  = M^T_tile[d, i].T @ W[d, e]   (node-major out)
  - relu fused into the PSUM->SBUF eviction on the scalar engine.

The aggregation runs CHUNK-MAJOR (one 512-wide output chunk at a time,
full contraction each): chunk k's hidden tiles finish at ~(k+1)/4 of the
layer, so AllGather k overlaps the remaining chunks' compute — only the
last AllGather is exposed at the layer boundary. The stationary H layout
in SBUF ([128, 512] pieces) is exactly what the chunked AllGathers
produce, so no transposes are needed anywhere.

PRECISION modes:
  - "fp32":   exact fp32 matmuls (4 cyc/row on the PE).
  - "split3": A and X/H split into bf16 hi+lo; aggregate computed as
              Ah@Xh + Al@Xh + Ah@Xl (3 bf16 passes, ~2.5e-6 rel err —
              fp32-class).
  - "bf16":   plain bf16 aggregate (1 cyc/row, half the DMA bytes,
              ~1.1e-3 rel err).
"""

import sys
from contextlib import ExitStack

if "/opt/trn_rl_repo" not in sys.path:
    sys.path.insert(0, "/opt/trn_rl_repo")

import numpy as np

N_NODES = 16384
D = 128
NCORES = 8
ROWS = N_NODES // NCORES  # 2048

PRECISION = "fp32"  # "fp32" | "split3" | "bf16"


def _geom(n_nodes=N_NODES, ncores=NCORES, precision=PRECISION):
    esz = 4 if precision == "fp32" else 2
    nsplit = 2 if precision == "split3" else 1  # hi/lo operand copies
    rows = n_nodes // ncores
    jt = n_nodes // 128          # total j-tiles (contraction tiles)
    jt_per_rank = jt // ncores   # j-tiles covered by one rank's nodes
    nh = 2 if rows >= 256 else 1  # column halves per layer
    hw = rows // nh              # half width
    ic = min(512, hw)            # i-chunk width (one PSUM bank, fp32 out)
    nch_h = hw // ic             # i-chunks per half
    # j-tiles per A DMA: ~2 MiB per transfer; deep buffer pool so the
    # A-stream prefetch covers the inter-layer AllGather window
    target = 2 * 1024 * 1024
    jg = max(1, target // (128 * hw * esz))
    jg = min(jg, jt)
    while jt % jg:
        jg -= 1
    a_bufs = {"bf16": 8, "fp32": 6, "split3": 3}[precision]
    # j-tile processing order: tiles whose hidden-state source lies in the
    # FIRST halves of every rank come first, so layer 1 can start on
    # AllGather-0 data while AllGather-1 is still in flight
    half_jt = max(1, jt_per_rank // nh)
    order = [j for j in range(jt) if (j % jt_per_rank) // half_jt == 0]
    order += [j for j in range(jt) if (j % jt_per_rank) // half_jt != 0]
    return dict(
        esz=esz, nsplit=nsplit, rows=rows, jt=jt, jt_per_rank=jt_per_rank,
        nh=nh, hw=hw, ic=ic, nch_h=nch_h, jg=jg, ndma_h=jt // jg,
        a_bufs=a_bufs, order=order, half_jt=half_jt,
    )


def build_gcn(n_nodes=N_NODES, d=D, ncores=NCORES, precision=PRECISION):
    """Build the SPMD Bass program (one program, runs on all cores)."""
    import concourse.bass as bass  # noqa: F401
    import concourse.tile as tile
    from concourse import bacc, mybir

    F32 = mybir.dt.float32
    BF16 = mybir.dt.bfloat16
    agg_dt = F32 if precision == "fp32" else BF16

    g_ = _geom(n_nodes, ncores, precision)
    nsplit, rows, jt = g_["nsplit"], g_["rows"], g_["jt"]
    jt_per_rank, ic, nch_h = g_["jt_per_rank"], g_["ic"], g_["nch_h"]
    nh, hw, half_jt = g_["nh"], g_["hw"], g_["half_jt"]
    jg, ndma_h, a_bufs = g_["jg"], g_["ndma_h"], g_["a_bufs"]
    order = g_["order"]
    lt = ic // 128               # linear i-tiles (and h tiles) per chunk

    nc = bacc.Bacc("TRN2", target_bir_lowering=False, num_devices=ncores)

    # A^T shards, host pre-tiled: DMA group (h, g) is the contiguous
    # block a_in[(h*ndma_h+g)*128 : +128, :] covering permuted j-tiles
    # order[g*jg : (g+1)*jg] x output columns [h*hw, (h+1)*hw)
    a_in = [
        nc.dram_tensor(
            f"a{s}", [nh * ndma_h * 128, jg * hw], agg_dt, kind="ExternalInput"
        )
        for s in range(nsplit)
    ]
    # x_t: X pre-tiled on host into the AllGather layout:
    # x_t[r*128 + p, tl*128 + dd] = X[(r*jt_per_rank + tl)*128 + p, dd]
    x_in = [
        nc.dram_tensor(f"x{s}", [ncores * 128, rows], agg_dt, kind="ExternalInput")
        for s in range(nsplit)
    ]
    w0 = nc.dram_tensor("w0", [d, d], F32, kind="ExternalInput")
    w1 = nc.dram_tensor("w1", [d, d], F32, kind="ExternalInput")
    h_out = nc.dram_tensor("h_out", [rows, d], F32, kind="ExternalOutput")

    relu = mybir.ActivationFunctionType.Relu

    with tile.TileContext(nc) as tc, ExitStack() as ctx:
        sb1 = ctx.enter_context(tc.tile_pool(name="sb1", bufs=1))
        stat_pool = ctx.enter_context(
            tc.tile_pool(name="stat", bufs=ncores * nh * nsplit)
        )
        a_pool = ctx.enter_context(tc.tile_pool(name="a", bufs=a_bufs))
        m_pool = ctx.enter_context(tc.tile_pool(name="m", bufs=2))
        h_pool = ctx.enter_context(tc.tile_pool(name="h", bufs=4))
        split_pool = ctx.enter_context(tc.tile_pool(name="spl", bufs=4))
        agg_pool = ctx.enter_context(tc.tile_pool(name="agg", bufs=4, space="PSUM"))
        lin_pool = ctx.enter_context(tc.tile_pool(name="lin", bufs=2, space="PSUM"))
        dram = ctx.enter_context(tc.tile_pool(name="dram", bufs=1, space="DRAM"))

        w0_sb = sb1.tile([d, d], F32)
        nc.scalar.dma_start(out=w0_sb[:], in_=w0[:])
        w1_sb = sb1.tile([d, d], F32)
        nc.scalar.dma_start(out=w1_sb[:], in_=w1[:])

        def load_stat_pieces(srcs, lname):
            """srcs[s][h]: [ncores*128, half_jt*128] DRAM views.
            Returns stat[s][r][h] = [128, half_jt*128] SBUF tile."""
            out = []
            for s in range(nsplit):
                per_rank = [[None] * nh for _ in range(ncores)]
                for h in range(nh):
                    for r in range(ncores):
                        sc = stat_pool.tile(
                            [128, half_jt * 128], agg_dt,
                            name=f"{lname}{s}_{r}_{h}", tag="sc",
                        )
                        nc.scalar.dma_start(
                            out=sc[:], in_=srcs[s][h][r * 128 : (r + 1) * 128, :]
                        )
                        per_rank[r][h] = sc
                out.append(per_rank)
            return out

        def layer(stat, w_sb, write_out, half_done):
            # stat[s][r][hs]: stationary pieces; j-tile j lives in piece
            # (r=j//jt_per_rank, hs=(j%jt_per_rank)//half_jt)
            passes = [(0, 0)] if nsplit == 1 else [(0, 0), (1, 0), (0, 1)]
            for h in range(nh):
                agg = [
                    agg_pool.tile([128, ic], F32, name=f"ps{cc}", tag="ps")
                    for cc in range(nch_h)
                ]
                for g in range(ndma_h):
                    ats = []
                    for s in range(nsplit):
                        at = a_pool.tile(
                            [128, jg * hw], agg_dt, name=f"at{s}", tag=f"at{s}"
                        )
                        nc.sync.dma_start(
                            out=at[:],
                            in_=a_in[s][
                                (h * ndma_h + g) * 128 : (h * ndma_h + g + 1) * 128,
                                :,
                            ],
                        )
                        ats.append(at)
                    for t in range(jg):
                        sq = g * jg + t
                        j = order[sq]
                        r, jr = j // jt_per_rank, j % jt_per_rank
                        hs, tl2 = jr // half_jt, jr % half_jt
                        for pi, (ls, rs) in enumerate(passes):
                            lhs = stat[ls][r][hs][:, tl2 * 128 : (tl2 + 1) * 128]
                            for cc in range(nch_h):
                                nc.tensor.matmul(
                                    agg[cc][:],
                                    lhsT=lhs,
                                    rhs=ats[rs][
                                        :, t * hw + cc * ic : t * hw + (cc + 1) * ic
                                    ],
                                    start=(sq == 0 and pi == 0),
                                    stop=(sq == jt - 1 and pi == len(passes) - 1),
                                )
                # linear + relu, node-major output tiles
                for cc in range(nch_h):
                    mt = m_pool.tile([128, ic], F32, name="mt", tag="mt")
                    nc.vector.tensor_copy(out=mt[:], in_=agg[cc][:])
                    for it in range(lt):
                        lp = lin_pool.tile([128, d], F32, name="lp", tag="lp")
                        nc.tensor.matmul(
                            lp[:],
                            lhsT=mt[:, it * 128 : (it + 1) * 128],
                            rhs=w_sb[:],
                            start=True,
                            stop=True,
                        )
                        ht = h_pool.tile([128, d], F32, name="ht", tag="ht")
                        nc.scalar.activation(ht[:], lp[:], relu)
                        write_out(h, cc * lt + it, ht)
                half_done(h)

        # ---- layer 0 ----
        hww = half_jt * 128  # stationary piece width (= hw)
        stat0 = load_stat_pieces(
            [
                [
                    x_in[s][:, h * hww : (h + 1) * hww]
                    for h in range(nh)
                ]
                for s in range(nsplit)
            ],
            "sx",
        )
        # per-half hidden-state bounce ([hi | lo] packed when split)
        h_tb = [
            dram.tile([128, nsplit * hww], agg_dt, name=f"h_tb{h}")
            for h in range(nh)
        ]
        h_ag = [
            dram.tile(
                [ncores * 128, nsplit * hww], agg_dt, addr_space="Shared",
                name=f"h_ag{h}",
            )
            for h in range(nh)
        ]

        def write_l0(h, tl, ht):
            if precision == "fp32":
                nc.scalar.dma_start(
                    out=h_tb[h][:, tl * 128 : (tl + 1) * 128], in_=ht[:]
                )
                return
            hh = split_pool.tile([128, d], BF16, name="hh", tag="hh")
            nc.vector.tensor_copy(out=hh[:], in_=ht[:])
            nc.scalar.dma_start(out=h_tb[h][:, tl * 128 : (tl + 1) * 128], in_=hh[:])
            if nsplit == 2:
                hh32 = split_pool.tile([128, d], F32, name="hh32", tag="hh32")
                nc.vector.tensor_copy(out=hh32[:], in_=hh[:])
                hl = split_pool.tile([128, d], BF16, name="hl", tag="hl")
                nc.vector.tensor_sub(out=hl[:], in0=ht[:], in1=hh32[:])
                nc.scalar.dma_start(
                    out=h_tb[h][:, hww + tl * 128 : hww + (tl + 1) * 128], in_=hl[:]
                )

        def ag_l0(h):
            import concourse.mybir as _mb

            nc.gpsimd.collective_compute(
                "AllGather",
                _mb.AluOpType.bypass,
                replica_groups=[list(range(ncores))],
                ins=[h_tb[h][:]],
                outs=[h_ag[h][:]],
            )

        layer(stat0, w0_sb, write_l0, ag_l0)

        # ---- layer 1 ----
        stat1 = load_stat_pieces(
            [
                [h_ag[h][:, s * hww : (s + 1) * hww] for h in range(nh)]
                for s in range(nsplit)
            ],
            "sh",
        )

        def write_l1(h, tl, ht):
            base = h * hw + tl * 128
            nc.scalar.dma_start(out=h_out[base : base + 128, :], in_=ht[:])

        layer(stat1, w1_sb, write_l1, lambda h: None)

    nc.finalize()
    return nc


def _tile_stat(X, ncores, jt_per_rank):
    rows = jt_per_rank * 128
    return np.ascontiguousarray(
        X.reshape(ncores, jt_per_rank, 128, D).transpose(0, 2, 1, 3)
        .reshape(ncores * 128, rows)
    )


def shard_inputs(A_norm, X, n_nodes=N_NODES, ncores=NCORES, precision=PRECISION):
    """Host-side shard prep. Returns per-core input maps."""
    import ml_dtypes

    bf16 = ml_dtypes.bfloat16
    g_ = _geom(n_nodes, ncores, precision)
    rows, jt_per_rank = g_["rows"], g_["jt_per_rank"]
    jg, ndma_h = g_["jg"], g_["ndma_h"]
    nh, hw, order = g_["nh"], g_["hw"], g_["order"]

    def tile_a(a_tc):
        # [n_nodes, rows] -> [nh*ndma_h*128, jg*hw]: DMA group (h, g) is
        # the contiguous block covering permuted j-tiles order[g*jg:(g+1)*jg]
        # x output columns [h*hw, (h+1)*hw)
        perm = a_tc.reshape(len(order), 128, nh, hw)[order]  # [jt,128,nh,hw]
        return np.ascontiguousarray(
            perm.reshape(ndma_h, jg, 128, nh, hw)
            .transpose(3, 0, 2, 1, 4)
            .reshape(nh * ndma_h * 128, jg * hw)
        )

    x_t = _tile_stat(X, ncores, jt_per_rank)
    if precision == "fp32":
        xs = [x_t]
    else:
        x_hi = x_t.astype(bf16)
        xs = [x_hi]
        if precision == "split3":
            xs.append((x_t - x_hi.astype(np.float32)).astype(bf16))

    in_maps = []
    for c in range(ncores):
        a_tc = np.ascontiguousarray(A_norm[c * rows : (c + 1) * rows, :].T)
        m = {}
        if precision == "fp32":
            m["a0"] = tile_a(a_tc)
        else:
            a_hi = a_tc.astype(bf16)
            m["a0"] = tile_a(a_hi)
            if precision == "split3":
                m["a1"] = tile_a((a_tc - a_hi.astype(np.float32)).astype(bf16))
        for s, x in enumerate(xs):
            m[f"x{s}"] = x
        in_maps.append(m)
    return in_maps


_CACHED = {}


def kernel(A_norm, X, W0, W1):
    A_norm = np.ascontiguousarray(A_norm, dtype=np.float32)
    X = np.ascontiguousarray(X, dtype=np.float32)
    W0 = np.ascontiguousarray(W0, dtype=np.float32)
    W1 = np.ascontiguousarray(W1, dtype=np.float32)

    from concourse.bass_utils import run_bass_kernel_spmd

    if PRECISION not in _CACHED:
        _CACHED[PRECISION] = build_gcn(precision=PRECISION)
    nc = _CACHED[PRECISION]

    in_maps = shard_inputs(A_norm, X, precision=PRECISION)
    for m in in_maps:
        m["w0"] = W0
        m["w1"] = W1

    res = run_bass_kernel_spmd(nc, in_maps, core_ids=list(range(NCORES)))
    return np.concatenate([res.results[c]["h_out"] for c in range(NCORES)], axis=0)


# revision 17
# speedup vs baseline: 1.0186x; 1.0186x over previous
"""2-layer dense GCN on 8 Trainium2 NeuronCores.

Reference computation (all fp32):
    H0 = relu((A_norm @ X) @ W0)
    H1 = relu((A_norm @ H0) @ W1)
A_norm: [16384, 16384], X: [16384, 128], W0/W1: [128, 128].

Sharding: 1D row partition of A_norm (2048 rows/core). Each core holds
A[rows_c].T (host-transposed so the node-contraction dim lands on SBUF
partitions), computes its row block of each layer, and the hidden state
is exchanged between layers with chunked on-device AllGathers.

Device layout is transpose-free:
  - aggregate:  psum[d, i] += X_tile[j, d].T @ A_T_tile[j, i]
                (lhsT = stationary node-major X/H tile, rhs = A^T slice)
  - linear:     psum[i, e]  = M^T_tile[d, i].T @ W[d, e]   (node-major out)
  - relu fused into the PSUM->SBUF eviction on the scalar engine.

The aggregation runs CHUNK-MAJOR (one 512-wide output chunk at a time,
full contraction each): chunk k's hidden tiles finish at ~(k+1)/4 of the
layer, so AllGather k overlaps the remaining chunks' compute — only the
last AllGather is exposed at the layer boundary. The stationary H layout
in SBUF ([128, 512] pieces) is exactly what the chunked AllGathers
produce, so no transposes are needed anywhere.

PRECISION modes:
  - "fp32":   exact fp32 matmuls (4 cyc/row on the PE).
  - "split3": A and X/H split into bf16 hi+lo; aggregate computed as
              Ah@Xh + Al@Xh + Ah@Xl (3 bf16 passes, ~2.5e-6 rel err —
              fp32-class).
  - "bf16":   plain bf16 aggregate (1 cyc/row, half the DMA bytes,
              ~1.1e-3 rel err).
"""

import sys
from contextlib import ExitStack

if "/opt/trn_rl_repo" not in sys.path:
    sys.path.insert(0, "/opt/trn_rl_repo")

import numpy as np

N_NODES = 16384
D = 128
NCORES = 8
ROWS = N_NODES // NCORES  # 2048

PRECISION = "bf16"  # "fp32" | "split3" | "bf16"


def _geom(n_nodes=N_NODES, ncores=NCORES, precision=PRECISION):
    esz = 4 if precision == "fp32" else 2
    nsplit = 2 if precision == "split3" else 1  # hi/lo operand copies
    rows = n_nodes // ncores
    jt = n_nodes // 128          # total j-tiles (contraction tiles)
    jt_per_rank = jt // ncores   # j-tiles covered by one rank's nodes
    ic = min(512, rows)          # i-chunk width (one PSUM bank, fp32 out)
    nch = rows // ic             # i-chunks per core
    # j-tiles per A DMA: ~2 MiB per transfer; deep buffer pool so the
    # A-stream prefetch covers the inter-layer AllGather window
    target = 2 * 1024 * 1024
    jg = max(1, target // (128 * rows * esz))
    jg = min(jg, jt)
    while jt % jg:
        jg -= 1
    a_bufs = {"bf16": 8, "fp32": 6, "split3": 3}[precision]
    return dict(
        esz=esz, nsplit=nsplit, rows=rows, jt=jt, jt_per_rank=jt_per_rank,
        ic=ic, nch=nch, jg=jg, ndma=jt // jg, a_bufs=a_bufs,
    )


def build_gcn(n_nodes=N_NODES, d=D, ncores=NCORES, precision=PRECISION):
    """Build the SPMD Bass program (one program, runs on all cores)."""
    import concourse.bass as bass  # noqa: F401
    import concourse.tile as tile
    from concourse import bacc, mybir

    F32 = mybir.dt.float32
    BF16 = mybir.dt.bfloat16
    agg_dt = F32 if precision == "fp32" else BF16

    g_ = _geom(n_nodes, ncores, precision)
    nsplit, rows, jt = g_["nsplit"], g_["rows"], g_["jt"]
    jt_per_rank, ic, nch = g_["jt_per_rank"], g_["ic"], g_["nch"]
    jg, ndma, a_bufs = g_["jg"], g_["ndma"], g_["a_bufs"]
    lt = ic // 128               # linear i-tiles (and h tiles) per chunk

    nc = bacc.Bacc("TRN2", target_bir_lowering=False, num_devices=ncores)

    # A^T shards, host pre-tiled: DMA group g is the contiguous block
    # a_in[g*128 : (g+1)*128, :], covering j-tiles [g*jg, (g+1)*jg) x
    # all output columns, with a_in[g*128+p, t*rows+i] = A^T[(g*jg+t)*128+p, i]
    a_in = [
        nc.dram_tensor(
            f"a{s}", [ndma * 128, jg * rows], agg_dt, kind="ExternalInput"
        )
        for s in range(nsplit)
    ]
    # x_t: X pre-tiled on host into the AllGather layout:
    # x_t[r*128 + p, tl*128 + dd] = X[(r*jt_per_rank + tl)*128 + p, dd]
    x_in = [
        nc.dram_tensor(f"x{s}", [ncores * 128, rows], agg_dt, kind="ExternalInput")
        for s in range(nsplit)
    ]
    w0 = nc.dram_tensor("w0", [d, d], F32, kind="ExternalInput")
    w1 = nc.dram_tensor("w1", [d, d], F32, kind="ExternalInput")
    h_out = nc.dram_tensor("h_out", [rows, d], F32, kind="ExternalOutput")

    relu = mybir.ActivationFunctionType.Relu

    with tile.TileContext(nc) as tc, ExitStack() as ctx:
        sb1 = ctx.enter_context(tc.tile_pool(name="sb1", bufs=1))
        stat_pool = ctx.enter_context(
            tc.tile_pool(name="stat", bufs=ncores * nsplit)
        )
        a_pool = ctx.enter_context(tc.tile_pool(name="a", bufs=a_bufs))
        m_pool = ctx.enter_context(tc.tile_pool(name="m", bufs=2))
        h_pool = ctx.enter_context(tc.tile_pool(name="h", bufs=4))
        split_pool = ctx.enter_context(tc.tile_pool(name="spl", bufs=4))
        agg_pool = ctx.enter_context(tc.tile_pool(name="agg", bufs=4, space="PSUM"))
        lin_pool = ctx.enter_context(tc.tile_pool(name="lin", bufs=2, space="PSUM"))
        dram = ctx.enter_context(tc.tile_pool(name="dram", bufs=1, space="DRAM"))

        w0_sb = sb1.tile([d, d], F32)
        nc.scalar.dma_start(out=w0_sb[:], in_=w0[:])
        w1_sb = sb1.tile([d, d], F32)
        nc.scalar.dma_start(out=w1_sb[:], in_=w1[:])

        def load_stat_chunks(srcs, lname):
            """srcs: per split s: [ncores*128, rows] DRAM view.
            Returns stat[s][r] = [128, rows] SBUF tile."""
            out = []
            for s in range(nsplit):
                chunks = []
                for r in range(ncores):
                    sc = stat_pool.tile(
                        [128, rows], agg_dt, name=f"{lname}{s}_{r}", tag="sc"
                    )
                    nc.scalar.dma_start(
                        out=sc[:], in_=srcs[s][r * 128 : (r + 1) * 128, :]
                    )
                    chunks.append(sc)
                out.append(chunks)
            return out

        def layer(stat, w_sb, write_out, layer_done):
            # stat[s][r]: stationary chunks; j-tile j lives in chunk
            # r=j//jt_per_rank at cols (j%jt_per_rank)*128
            passes = [(0, 0)] if nsplit == 1 else [(0, 0), (1, 0), (0, 1)]
            agg = [
                agg_pool.tile([128, ic], F32, name=f"ps{c}", tag="ps")
                for c in range(nch)
            ]
            for g in range(ndma):
                ats = []
                for s in range(nsplit):
                    at = a_pool.tile(
                        [128, jg * rows], agg_dt, name=f"at{s}", tag=f"at{s}"
                    )
                    nc.sync.dma_start(
                        out=at[:], in_=a_in[s][g * 128 : (g + 1) * 128, :]
                    )
                    ats.append(at)
                for t in range(jg):
                    j = g * jg + t
                    jr = j % jt_per_rank
                    for pi, (ls, rs) in enumerate(passes):
                        lhs = stat[ls][j // jt_per_rank][
                            :, jr * 128 : (jr + 1) * 128
                        ]
                        for c in range(nch):
                            nc.tensor.matmul(
                                agg[c][:],
                                lhsT=lhs,
                                rhs=ats[rs][
                                    :, t * rows + c * ic : t * rows + (c + 1) * ic
                                ],
                                start=(j == 0 and pi == 0),
                                stop=(j == jt - 1 and pi == len(passes) - 1),
                            )
            # linear + relu, node-major output tiles
            for c in range(nch):
                mt = m_pool.tile([128, ic], F32, name="mt", tag="mt")
                nc.vector.tensor_copy(out=mt[:], in_=agg[c][:])
                for it in range(lt):
                    lp = lin_pool.tile([128, d], F32, name="lp", tag="lp")
                    nc.tensor.matmul(
                        lp[:],
                        lhsT=mt[:, it * 128 : (it + 1) * 128],
                        rhs=w_sb[:],
                        start=True,
                        stop=True,
                    )
                    ht = h_pool.tile([128, d], F32, name="ht", tag="ht")
                    nc.scalar.activation(ht[:], lp[:], relu)
                    write_out(c, it, ht)
            layer_done()

        # ---- layer 0 ----
        stat0 = load_stat_chunks([x[:] for x in x_in], "sx")
        # packed hidden-state bounce ([hi | lo] along free dim when split)
        h_tb = dram.tile([128, nsplit * rows], agg_dt, name="h_tb")
        h_ag = dram.tile(
            [ncores * 128, nsplit * rows], agg_dt, addr_space="Shared", name="h_ag"
        )

        def write_l0(c, it, ht):
            tl = c * lt + it
            if precision == "fp32":
                nc.scalar.dma_start(
                    out=h_tb[:, tl * 128 : (tl + 1) * 128], in_=ht[:]
                )
                return
            hh = split_pool.tile([128, d], BF16, name="hh", tag="hh")
            nc.vector.tensor_copy(out=hh[:], in_=ht[:])
            nc.scalar.dma_start(out=h_tb[:, tl * 128 : (tl + 1) * 128], in_=hh[:])
            if nsplit == 2:
                hh32 = split_pool.tile([128, d], F32, name="hh32", tag="hh32")
                nc.vector.tensor_copy(out=hh32[:], in_=hh[:])
                hl = split_pool.tile([128, d], BF16, name="hl", tag="hl")
                nc.vector.tensor_sub(out=hl[:], in0=ht[:], in1=hh32[:])
                nc.scalar.dma_start(
                    out=h_tb[:, rows + tl * 128 : rows + (tl + 1) * 128], in_=hl[:]
                )

        def ag_l0():
            import concourse.mybir as _mb

            nc.gpsimd.collective_compute(
                "AllGather",
                _mb.AluOpType.bypass,
                replica_groups=[list(range(ncores))],
                ins=[h_tb[:]],
                outs=[h_ag[:]],
            )

        layer(stat0, w0_sb, write_l0, ag_l0)

        # ---- layer 1 ----
        stat1 = load_stat_chunks(
            [h_ag[:, s * rows : (s + 1) * rows] for s in range(nsplit)], "sh"
        )

        def write_l1(c, it, ht):
            nc.scalar.dma_start(
                out=h_out[c * ic + it * 128 : c * ic + (it + 1) * 128, :], in_=ht[:]
            )

        layer(stat1, w1_sb, write_l1, lambda: None)

    nc.finalize()
    return nc


def _tile_stat(X, ncores, jt_per_rank):
    rows = jt_per_rank * 128
    return np.ascontiguousarray(
        X.reshape(ncores, jt_per_rank, 128, D).transpose(0, 2, 1, 3)
        .reshape(ncores * 128, rows)
    )


def shard_inputs(A_norm, X, n_nodes=N_NODES, ncores=NCORES, precision=PRECISION):
    """Host-side shard prep. Returns per-core input maps."""
    import ml_dtypes

    bf16 = ml_dtypes.bfloat16
    g_ = _geom(n_nodes, ncores, precision)
    rows, jt_per_rank = g_["rows"], g_["jt_per_rank"]
    jg, ndma = g_["jg"], g_["ndma"]

    def tile_a(a_tc):
        # [n_nodes, rows] -> [ndma*128, jg*rows] so DMA group g is the
        # contiguous block a_pre[g*128:(g+1)*128, :] with
        # a_pre[g*128+p, t*rows+i] = a_tc[(g*jg+t)*128+p, i]
        return np.ascontiguousarray(
            a_tc.reshape(ndma, jg, 128, rows).swapaxes(1, 2)
            .reshape(ndma * 128, jg * rows)
        )

    x_t = _tile_stat(X, ncores, jt_per_rank)
    if precision == "fp32":
        xs = [x_t]
    else:
        x_hi = x_t.astype(bf16)
        xs = [x_hi]
        if precision == "split3":
            xs.append((x_t - x_hi.astype(np.float32)).astype(bf16))

    in_maps = []
    for c in range(ncores):
        a_tc = np.ascontiguousarray(A_norm[c * rows : (c + 1) * rows, :].T)
        m = {}
        if precision == "fp32":
            m["a0"] = tile_a(a_tc)
        else:
            a_hi = a_tc.astype(bf16)
            m["a0"] = tile_a(a_hi)
            if precision == "split3":
                m["a1"] = tile_a((a_tc - a_hi.astype(np.float32)).astype(bf16))
        for s, x in enumerate(xs):
            m[f"x{s}"] = x
        in_maps.append(m)
    return in_maps


_CACHED = {}


def kernel(A_norm, X, W0, W1):
    A_norm = np.ascontiguousarray(A_norm, dtype=np.float32)
    X = np.ascontiguousarray(X, dtype=np.float32)
    W0 = np.ascontiguousarray(W0, dtype=np.float32)
    W1 = np.ascontiguousarray(W1, dtype=np.float32)

    from concourse.bass_utils import run_bass_kernel_spmd

    if PRECISION not in _CACHED:
        _CACHED[PRECISION] = build_gcn(precision=PRECISION)
    nc = _CACHED[PRECISION]

    in_maps = shard_inputs(A_norm, X, precision=PRECISION)
    for m in in_maps:
        m["w0"] = W0
        m["w1"] = W1

    res = run_bass_kernel_spmd(nc, in_maps, core_ids=list(range(NCORES)))
    return np.concatenate([res.results[c]["h_out"] for c in range(NCORES)], axis=0)
